# revision 20
# baseline (speedup 1.0000x reference)
"""Multi-head attention (B=2,S=2048,D=1024,H=16) on 8 trn2 NeuronCores.

Sharding: core = b*4 + g  (b = batch 0..1, g = head-group 0..3, 4 heads each).
Each core computes QKV projections for its 256 output dims, causal attention
for its 4 heads (scores kept transposed: [s_k, s_q]), and a K-sliced partial
of the output projection (transposed: [D, S]).  Host sums the 4 partials per
batch and adds b_o.

All matmuls in bf16 (fp32 PSUM accumulate); softmax without max-subtraction
(scores/8 are small, exp cannot overflow); sumexp via an all-ones [128,64]
stationary matmul that also broadcasts the sum to all partitions of each
head's half, so normalization is a plain elementwise multiply.

Pipeline structure: coarse per-quarter input DMAs spread over the sync /
scalar / gpsimd issue queues; PSUM split into dedicated pools (2x score
double-buffer, 2x projection, ctx, sumexp) so the score->exp->PV stream,
the woven QKV/O projections, and the accumulators never serialize through
a shared slot ring.  The last quarter's output projection is split per
head-pair (partial to a second output tensor summed on the host) to
shorten the kernel tail.
"""
import sys

if "/opt/trn_rl_repo" not in sys.path:
    sys.path.insert(0, "/opt/trn_rl_repo")

import numpy as np
import ml_dtypes

B, S, D, H = 2, 2048, 1024, 16
HD = D // H            # 64
G = 4                  # head groups (one per core within a batch)
HPG = H // G           # 4 heads per group
DG = HPG * HD          # 256 dims per group
SCALE = 8.0
NCORES = 8
NQC = S // 512         # 4 query chunks
NJ = S // 128          # 16 key tiles
KC = D // 128          # 8 contraction chunks
BF16 = ml_dtypes.bfloat16

_CACHE = {}


def _build(causal: bool):
    import concourse.mybir as mybir
    import concourse.tile as tile
    from concourse import bacc

    f32 = mybir.dt.float32
    b16 = mybir.dt.bfloat16
    Exp = mybir.ActivationFunctionType.Exp

    nc = bacc.Bacc(None, target_bir_lowering=False)

    # inputs host-prepacked quarter-major per partition: qP[p, n*KC*512 +
    # kc*512 + s] = q^T[kc*128 + p, n*512 + s] -- each quarter DMA is then a
    # plain 2D copy with 8KB-contiguous per-partition runs (fat DMA packets,
    # full HBM rate; strided 1KB packets lose the SDMA round-robin 4:1)
    qP = nc.dram_tensor("qP", [128, NQC * KC * 512], b16, kind="ExternalInput")
    kP = nc.dram_tensor("kP", [128, NQC * KC * 512], b16, kind="ExternalInput")
    vP = nc.dram_tensor("vP", [128, NQC * KC * 512], b16, kind="ExternalInput")
    # weights host-prepacked to the exact SBUF tile layout (one DMA each)
    wqT = nc.dram_tensor("wqT", [128, KC * DG], b16, kind="ExternalInput")
    wkT = nc.dram_tensor("wkT", [128, KC * DG], b16, kind="ExternalInput")
    wvT = nc.dram_tensor("wvT", [128, KC * DG], b16, kind="ExternalInput")
    woT = nc.dram_tensor("woT", [128, 2 * D], b16, kind="ExternalInput")
    bq = nc.dram_tensor("bq", [128, 2], f32, kind="ExternalInput")
    bk = nc.dram_tensor("bk", [128, 2], f32, kind="ExternalInput")
    bv128 = nc.dram_tensor("bv128", [128, DG], b16, kind="ExternalInput")
    tri = nc.dram_tensor("tri", [128, 128], b16, kind="ExternalInput")
    # output packed as contiguous (quarter, row-block) tiles so each
    # quarter's result leaves in ONE fat DMA: outP[p, (c*KC+dc)*512 + s] =
    # out[dc*128+p, c*512+s]; host unpacks (free)
    outP = nc.dram_tensor("outP", [128, NQC * KC * 512], b16, kind="ExternalOutput")
    # pair-0 partial of the last quarter's output projection (host adds it)
    out2P = nc.dram_tensor("out2P", [128, KC * 512], b16, kind="ExternalOutput")

    with tile.TileContext(nc) as tc:
        with (
            tc.tile_pool(name="consts", bufs=1) as consts,
            tc.tile_pool(name="proj", bufs=1) as proj,
            tc.tile_pool(name="pin", bufs=1) as pin,
            tc.tile_pool(name="probs", bufs=8) as probsp,
            tc.tile_pool(name="rec", bufs=2) as recp,
            tc.tile_pool(name="ost", bufs=1) as ostp,
            tc.tile_pool(name="scp", bufs=2, space="PSUM") as scp,
            tc.tile_pool(name="pp", bufs=2, space="PSUM") as ppool,
            tc.tile_pool(name="cpsum", bufs=1, space="PSUM") as cpsum,
            tc.tile_pool(name="upsum", bufs=1, space="PSUM") as upsum,
        ):
            # --- constant tiles -------------------------------------------
            wq_t = consts.tile([128, KC * DG], b16)
            wk_t = consts.tile([128, KC * DG], b16)
            wv_t = consts.tile([128, KC * DG], b16)
            wo_t = consts.tile([128, 2 * D], b16)
            bq_t = consts.tile([128, 2], f32)
            bk_t = consts.tile([128, 2], f32)
            bv_t = consts.tile([128, DG], b16)
            tri_t = consts.tile([128, 128], b16)
            ones64_t = consts.tile([128, HD], b16)
            nc.vector.memset(ones64_t[:], 1.0)
            warm_sb = consts.tile([128, 128], b16)
            nc.vector.memset(warm_sb[:], 0.0)

            # --- persistent projection outputs ----------------------------
            # qpT/kpT: pair p in cols [p*S,(p+1)*S); rows 0:64 head 2p, 64:128 head 2p+1
            qpT = proj.tile([128, 2 * S], b16)
            kpT = proj.tile([128, 2 * S], b16)
            # vp: key tile j in cols [j*DG,(j+1)*DG); within: local head hh at 64*hh
            vp = proj.tile([128, NJ * DG], b16)
            # ctxT: same pair layout as qpT, normalized attention output (c x s)
            ctxT = proj.tile([128, 2 * S], b16)

            # --- per-quarter input tiles + coarse DMA schedule ------------
            # quarter n of q/k/v lives in one tile, contraction chunk kc at
            # cols [kc*512,(kc+1)*512); one 1MB DMA per (tensor, quarter),
            # issued across three queues (sync / scalar / gpsimd) so first
            # bytes land fast and the HW rings stream at full HBM rate
            qin = [pin.tile([128, KC * 512], b16, name=f"qin{n}") for n in range(NQC)]
            kin = [pin.tile([128, KC * 512], b16, name=f"kin{n}") for n in range(NQC)]
            vin = [pin.tile([128, KC * 512], b16, name=f"vin{n}") for n in range(NQC)]

            def dma_quarter(eng, dst, srcP, n, split=1):
                w = KC * 512 // split
                for h in range(split):
                    eng.dma_start(dst[:, h * w:(h + 1) * w],
                                  srcP[:, n * KC * 512 + h * w: n * KC * 512 + (h + 1) * w])

            # ramp-critical transfers alternate across BOTH rings in
            # dependency-need order (each ring sustains only ~half the HBM
            # rate when both stream): wq+wk land together, then q0+k0, then
            # wv+v0.  The scalar queue carries NO DMAs: a blocked issue
            # there would stall the exp stream.
            nc.sync.dma_start(wq_t[:], wqT[:])
            nc.scalar.dma_start(wk_t[:], wkT[:])
            nc.gpsimd.dma_start(wv_t[:], wvT[:])
            dma_quarter(nc.sync, qin[0], qP, 0, split=2)
            dma_quarter(nc.scalar, kin[0], kP, 0, split=2)
            dma_quarter(nc.gpsimd, vin[0], vP, 0)
            nc.gpsimd.dma_start(bv_t[:], bv128[:])
            nc.gpsimd.dma_start(tri_t[:], tri[:])
            nc.gpsimd.dma_start(bq_t[:], bq[:])
            nc.gpsimd.dma_start(bk_t[:], bk[:])
            nc.sync.dma_start(wo_t[:], woT[:])
            for n in range(1, NQC):
                dma_quarter(nc.sync, qin[n], qP, n)
                dma_quarter(nc.sync, kin[n], kP, n)
                dma_quarter(nc.gpsimd, vin[n], vP, n)

            # warmup burst: keeps the PE activity monitor at full clock
            # while the first input quarters stream in
            warm_ps = scp.tile([128, 1024], f32, tag="sc", name="warm")
            for wi in range(64):
                nc.tensor.matmul(warm_ps[:, 0:128], warm_sb[:], warm_sb[:],
                                 start=(wi == 0), stop=(wi == 63))
            # preload the exp spline tables (~2.7us) during the DMA window
            nc.scalar.activation(warm_sb[:, 0:1], warm_sb[:, 0:1], Exp)

            # --- projections interleaved with attention, per quarter ------

            def qk_proj_m(name, src_n, w_t, dst, bias_t, n, m):
                ps = ppool.tile([128, 512], f32, tag="pp", name=f"{name}ps{m}{n}")
                for kc in range(KC):
                    for tl in (0, 64):
                        nc.tensor.matmul(
                            ps[tl:tl + 64, :],
                            w_t[:, kc * DG + m * 128 + tl: kc * DG + m * 128 + tl + 64],
                            src_n[:, kc * 512:(kc + 1) * 512],
                            start=(kc == 0), stop=(kc == KC - 1),
                            skip_group_check=True,
                        )
                nc.vector.tensor_scalar_add(
                    dst[:, m * S + n * 512: m * S + (n + 1) * 512],
                    ps, bias_t[:, m:m + 1],
                )

            def qk_proj(name, srcs, w_t, dst, bias_t, n):
                for m in range(2):
                    qk_proj_m(name, srcs[n], w_t, dst, bias_t, n, m)

            def v_proj_j(j):
                n = j // 4
                jj = j - 4 * n
                ps = ppool.tile([128, 512], f32, tag="pp", name=f"vps{j}")
                # token dim split into two 64-col stationaries at col
                # groups (0,0)/(0,64): disjoint PSUM partitions, one shared
                # 256-wide stream each -> both halves run concurrently and
                # the LDWEIGHTS hides across col groups (a full-array N=256
                # chain paces at LDW+stream = 2x slower)
                for kc in range(KC):
                    cs = kc * 512 + jj * 128
                    for tl in (0, 64):
                        nc.tensor.matmul(
                            ps[tl:tl + 64, 0:DG],
                            vin[n][:, cs + tl: cs + tl + 64],
                            wv_t[:, kc * DG:(kc + 1) * DG],
                            start=(kc == 0), stop=(kc == KC - 1),
                            skip_group_check=True,
                        )
                # bias folded into the PSUM->SBUF move (the 1-partition bias
                # matmul forced a 32x128 tiling mode switch = PE drain)
                nc.vector.tensor_add(vp[:, j * DG:(j + 1) * DG], ps[:, 0:DG], bv_t[:])

            def v_proj(n):
                for j in range(4 * n, 4 * n + 4):
                    v_proj_j(j)

            def attn_j_sc(c, p, j, nj):
                qoff = p * S + c * 512
                d = j - 4 * c if causal else -1
                coff = 0 if d < 0 else 128 * d
                sc = scp.tile([128, 1024], f32, tag="sc", name=f"sc{c}{p}{j}")
                for hh, (rlo, rhi) in enumerate(((0, 64), (64, 128))):
                    nc.tensor.matmul(
                        sc[:, hh * 512 + coff: hh * 512 + 512],
                        kpT[rlo:rhi, p * S + j * 128: p * S + (j + 1) * 128],
                        qpT[rlo:rhi, qoff + coff: qoff + 512],
                        start=True, stop=True, tile_position=(rlo, 0),
                    )
                pr = probsp.tile([128, 1024], b16, tag="pr", name=f"pr{c}{p}{j}")
                if coff == 0:
                    nc.scalar.activation(pr[:, 0:1024], sc[:, 0:1024], Exp, scale=1.0 / SCALE)
                else:
                    sc_v = sc.rearrange("p (h n) -> p h n", h=2)[:, :, coff:512]
                    pr_v = pr.rearrange("p (h n) -> p h n", h=2)[:, :, coff:512]
                    nc.scalar.activation(pr_v, sc_v, Exp, scale=1.0 / SCALE)
                if d >= 0:
                    for hh in range(2):
                        band = pr[:, hh * 512 + coff: hh * 512 + coff + 128]
                        nc.vector.tensor_mul(band, band, tri_t[:])
                return pr

            def attn_j_pv(c, p, j, nj, ctx_ps, sum_ps, pr):
                d = j - 4 * c if causal else -1
                coff = 0 if d < 0 else 128 * d
                first, last = (j == 0), (j == nj - 1)
                for hh in range(2):
                    prh = pr[:, hh * 512 + coff: hh * 512 + 512]
                    nc.tensor.matmul(
                        ctx_ps[hh * 64:(hh + 1) * 64, coff:512],
                        vp[:, j * DG + p * 128 + hh * 64: j * DG + p * 128 + (hh + 1) * 64],
                        prh, start=first, stop=last,
                        tile_position=(0, hh * 64), skip_group_check=True,
                    )
                for hh in range(2):
                    prh = pr[:, hh * 512 + coff: hh * 512 + 512]
                    nc.tensor.matmul(
                        sum_ps[hh * 64:(hh + 1) * 64, coff:512],
                        ones64_t[:], prh, start=first, stop=last,
                        tile_position=(0, hh * 64), skip_group_check=True,
                    )

            def attn_j(c, p, j, nj, ctx_ps, sum_ps):
                pr = attn_j_sc(c, p, j, nj)
                attn_j_pv(c, p, j, nj, ctx_ps, sum_ps, pr)

            def attn_pair(c, p, nj, j_lo, j_hi, ctx_ps, sum_ps, bg=None, grp=4):
                # process j-tiles in groups: all scores (64x128 tiling mode)
                # then all PV+sum matmuls (128x64 mode) -- each mode change
                # drains the PE, so batching by mode cuts the switch count
                # ~4x.  bg: zero-arg projection emitters (128x128 mode)
                # woven at group boundaries to fill PE slack under the
                # ACT-paced softmax.
                bg = list(bg or [])
                js = list(range(j_lo, j_hi))
                n_groups = (len(js) + grp - 1) // grp
                for gi in range(n_groups):
                    gj = js[gi * grp:(gi + 1) * grp]
                    prs = [attn_j_sc(c, p, j, nj) for j in gj]
                    for j, pr in zip(gj, prs):
                        attn_j_pv(c, p, j, nj, ctx_ps, sum_ps, pr)
                    take = len(bg) // (n_groups - gi) if gi < n_groups - 1 else 0
                    for _ in range(take):
                        bg.pop(0)()
                while bg:
                    bg.pop(0)()

            def norm_pair(c, p, ctx_ps, sum_ps):
                rc_t = recp.tile([128, 512], f32, tag="rc", name=f"rc{c}{p}")
                nc.vector.reciprocal_approx_fast(rc_t[:], sum_ps[:])
                nc.vector.tensor_mul(ctxT[:, p * S + c * 512: p * S + (c + 1) * 512], ctx_ps[:], rc_t[:])

            ostage = {}

            def oproj_dc(c, dc):
                ops = ppool.tile([128, 512], f32, tag="pp", name=f"op{c}{dc}")
                for p2 in range(2):
                    for tl in (0, 64):
                        nc.tensor.matmul(
                            ops[tl:tl + 64, :],
                            wo_t[:, p2 * D + dc * 128 + tl: p2 * D + dc * 128 + tl + 64],
                            ctxT[:, p2 * S + c * 512: p2 * S + (c + 1) * 512],
                            start=(p2 == 0), stop=(p2 == 1),
                            skip_group_check=True,
                        )
                if dc == 0:
                    ostage[c] = ostp.tile([128, KC * 512], b16, tag="ostage", bufs=2, name=f"ost{c}")
                nc.vector.tensor_copy(ostage[c][:, dc * 512:(dc + 1) * 512], ops)
                if dc == KC - 1:
                    nc.sync.dma_start(outP[:, c * KC * 512:(c + 1) * KC * 512], ostage[c][:])

            # tail pools rotation: at the end of the kernel the attention
            # psum pools are free, so the last 8 oproj matmuls each get
            # their own slot and never wait on the staging copies
            _tailp = [(ppool, "pp"), (scp, "sc"), (cpsum, "ctx"), (upsum, "sum")]

            def oproj_half_dc(c, dc, p2, use_act=False):
                # one-pair partial of the output projection for quarter c.
                # p2==0 goes to out2P (host adds); p2==1 goes to outP.
                if p2 == 1:
                    pool, ptag = _tailp[dc % 4]
                else:
                    pool, ptag = ppool, "pp"
                ops = pool.tile([128, 512], f32, tag=ptag, name=f"oh{c}{dc}{p2}")
                for tl in (0, 64):
                    nc.tensor.matmul(
                        ops[tl:tl + 64, :],
                        wo_t[:, p2 * D + dc * 128 + tl: p2 * D + dc * 128 + tl + 64],
                        ctxT[:, p2 * S + c * 512: p2 * S + (c + 1) * 512],
                        start=True, stop=True,
                        skip_group_check=True,
                    )
                if p2 == 0:
                    if dc == 0:
                        ostage["o2"] = ostp.tile([128, KC * 512], b16, tag="ost2", name="ost2")
                    nc.vector.tensor_copy(ostage["o2"][:, dc * 512:(dc + 1) * 512], ops)
                    if dc == KC - 1:
                        nc.sync.dma_start(out2P[:], ostage["o2"][:])
                else:
                    if dc == 0:
                        ostage[c] = ostp.tile([128, KC * 512], b16, tag="ostage", bufs=2, name=f"ost{c}")
                    dst = ostage[c][:, dc * 512:(dc + 1) * 512]
                    if use_act:
                        nc.scalar.copy(dst, ops)
                    else:
                        nc.vector.tensor_copy(dst, ops)
                    half = KC * 512 // 2
                    if dc == KC // 2 - 1:
                        nc.sync.dma_start(outP[:, c * KC * 512: c * KC * 512 + half],
                                          ostage[c][:, 0:half])
                    elif dc == KC - 1:
                        nc.sync.dma_start(outP[:, c * KC * 512 + half:(c + 1) * KC * 512],
                                          ostage[c][:, half:])

            if not causal:
                # no diagonal structure to pipeline against: project all
                # quarters upfront
                qk_proj("q", qin, wq_t, qpT, bq_t, 0)
                for n in range(NQC):
                    if n > 0:
                        qk_proj("q", qin, wq_t, qpT, bq_t, n)
                    qk_proj("k", kin, wk_t, kpT, bk_t, n)
                    v_proj(n)
            for c in range(NQC):
                nj = 4 * c + 4 if causal else NJ
                ctx0 = cpsum.tile([128, 512], f32, tag="ctx", name=f"ctx{c}0")
                sum0 = upsum.tile([128, 512], f32, tag="sum", name=f"sum{c}0")
                if causal and c == 0:
                    # ramp: q/k projections and ALL c=0 scores first (they
                    # need only the q/k quarter-0 DMAs), v projections after
                    # (the v0 DMA lands while the exp stream spins up)
                    qk_proj_m("q", qin[0], wq_t, qpT, bq_t, 0, 0)
                    warm2 = scp.tile([128, 1024], f32, tag="sc", name="warm2")
                    for wi in range(24):
                        nc.tensor.matmul(warm2[:, 0:128], warm_sb[:], warm_sb[:],
                                         start=(wi == 0), stop=(wi == 23))
                    qk_proj_m("k", kin[0], wk_t, kpT, bk_t, 0, 0)
                    prs0 = [attn_j_sc(0, 0, j, nj) for j in range(nj)]
                    qk_proj_m("q", qin[0], wq_t, qpT, bq_t, 0, 1)
                    qk_proj_m("k", kin[0], wk_t, kpT, bk_t, 0, 1)
                    prs1 = [attn_j_sc(0, 1, j, nj) for j in range(nj)]
                    v_proj(0)
                    for j in range(nj):
                        attn_j_pv(0, 0, j, nj, ctx0, sum0, prs0[j])
                    norm_pair(0, 0, ctx0, sum0)
                    ctx1 = cpsum.tile([128, 512], f32, tag="ctx", name="ctx01")
                    sum1 = upsum.tile([128, 512], f32, tag="sum", name="sum01")
                    for j in range(nj):
                        attn_j_pv(0, 1, j, nj, ctx1, sum1, prs1[j])
                    qk_proj_m("q", qin[1], wq_t, qpT, bq_t, 1, 0)
                    qk_proj_m("q", qin[1], wq_t, qpT, bq_t, 1, 1)
                    norm_pair(0, 1, ctx1, sum1)
                    continue
                # part A (earlier-quarter key tiles; needs only qpT of this
                # quarter) with this quarter's k/v projections woven between
                # slots so they retire before the diagonal part B needs them
                bg_a = []
                if causal:
                    bg_a += [lambda j=j: v_proj_j(j) for j in range(4 * c, 4 * c + 4)]
                    bg_a += [lambda m=m: qk_proj_m("k", kin[c], wk_t, kpT, bk_t, c, m) for m in range(2)]
                attn_pair(c, 0, nj, 0, min(4 * c, nj), ctx0, sum0, bg=bg_a)
                attn_pair(c, 0, nj, min(4 * c, nj), nj, ctx0, sum0)
                norm_pair(c, 0, ctx0, sum0)
                # pair 1 hosts: previous chunk's output projection, the next
                # quarter's q projection, and (for the last quarter) the
                # pair-0 half of its own output projection
                bg_b = []
                if c > 0:
                    bg_b += [lambda dc=dc: oproj_dc(c - 1, dc) for dc in range(KC)]
                if causal and c + 1 < NQC:
                    bg_b += [lambda m=m: qk_proj_m("q", qin[c + 1], wq_t, qpT, bq_t, c + 1, m) for m in range(2)]
                if c == NQC - 1:
                    bg_b += [lambda dc=dc: oproj_half_dc(c, dc, 0) for dc in range(KC)]
                ctx1 = cpsum.tile([128, 512], f32, tag="ctx", name=f"ctx{c}1")
                sum1 = upsum.tile([128, 512], f32, tag="sum", name=f"sum{c}1")
                attn_pair(c, 1, nj, 0, nj, ctx1, sum1, bg=bg_b)
                norm_pair(c, 1, ctx1, sum1)
            # tail: only the pair-1 half of the last quarter's output
            # projection remains; copies alternate DVE/ACT (ACT is idle now)
            for dc in range(KC):
                oproj_half_dc(NQC - 1, dc, 1, use_act=(dc % 2 == 1))

    nc.compile()
    return nc


def _get_nc(causal: bool):
    if causal not in _CACHE:
        _CACHE[causal] = _build(causal)
    return _CACHE[causal]


def _pack_w(w):
    # [D, DG] -> SBUF layout [128, KC*DG]: chunk kc of 128 rows side by side
    return np.ascontiguousarray(w.reshape(KC, 128, DG).transpose(1, 0, 2).reshape(128, KC * DG)).astype(BF16)


def _pack_in(x):
    # x [S, D] -> xP [128, NQC*KC*512]: xP[p, (n, kc, s)] = x^T[kc*128+p, n*512+s]
    xT = x.T.reshape(KC, 128, NQC, 512)
    return np.ascontiguousarray(xT.transpose(1, 2, 0, 3).reshape(128, NQC * KC * 512)).astype(BF16)


def make_in_maps(q, k, v, w_q, b_q, w_k, b_k, w_v, b_v, w_o):
    tri_keep = (np.arange(128)[:, None] <= np.arange(128)[None, :]).astype(BF16)
    qPn = [_pack_in(q[b]) for b in range(B)]
    kPn = [_pack_in(k[b]) for b in range(B)]
    vPn = [_pack_in(v[b]) for b in range(B)]
    in_maps = []
    for core in range(NCORES):
        b, g = core // G, core % G
        sl = slice(g * DG, (g + 1) * DG)
        woTg = np.ascontiguousarray(w_o[:, sl].T)  # [DG, D]
        in_maps.append({
            "qP": qPn[b], "kP": kPn[b], "vP": vPn[b],
            "wqT": _pack_w(np.ascontiguousarray(w_q[sl, :].T)),
            "wkT": _pack_w(np.ascontiguousarray(w_k[sl, :].T)),
            "wvT": _pack_w(np.ascontiguousarray(w_v[sl, :].T)),
            "woT": np.ascontiguousarray(
                woTg.reshape(2, 128, D).transpose(1, 0, 2).reshape(128, 2 * D)).astype(BF16),
            "bq": np.ascontiguousarray(b_q[sl].reshape(2, 128).T).astype(np.float32),
            "bk": np.ascontiguousarray(b_k[sl].reshape(2, 128).T).astype(np.float32),
            "bv128": np.ascontiguousarray(np.broadcast_to(b_v[None, sl], (128, DG))).astype(BF16),
            "tri": tri_keep,
        })
    return in_maps


def _reference_numpy(q, k, v, mask, w_q, b_q, w_k, b_k, w_v, b_v, w_o, b_o):
    qp = q @ w_q.T + b_q
    kp = k @ w_k.T + b_k
    vv = v @ w_v.T + b_v
    qp = qp.reshape(B, S, H, HD).transpose(0, 2, 1, 3)
    kp = kp.reshape(B, S, H, HD).transpose(0, 2, 1, 3)
    vv = vv.reshape(B, S, H, HD).transpose(0, 2, 1, 3)
    score = np.einsum("bhqd,bhkd->bhqk", qp, kp) / SCALE
    score = np.where(mask, -1e9, score)
    score -= score.max(axis=-1, keepdims=True)
    e = np.exp(score)
    attn = e / e.sum(axis=-1, keepdims=True)
    ctx = np.einsum("bhqk,bhkd->bhqd", attn, vv)
    ctx = ctx.transpose(0, 2, 1, 3).reshape(B, S, D)
    return (ctx @ w_o.T + b_o).astype(np.float32)


def kernel(q, k, v, mask, w_q, b_q, w_k, b_k, w_v, b_v, w_o, b_o):
    from concourse.bass_utils import run_bass_kernel_spmd

    q, k, v = (np.asarray(x, np.float32) for x in (q, k, v))
    mask = np.asarray(mask)
    causal_ref = np.triu(np.ones((S, S), bool), k=1)
    causal = all(np.array_equal(mask[b, 0], causal_ref) for b in range(B))
    if not causal and mask.any():
        # Unexpected mask pattern: fall back to exact numpy (never hit in
        # practice -- setup_inputs always builds the causal mask).
        return _reference_numpy(q, k, v, mask, w_q, b_q, w_k, b_k, w_v, b_v, w_o, b_o)

    nc = _get_nc(causal)
    in_maps = make_in_maps(q, k, v, w_q, b_q, w_k, b_k, w_v, b_v, w_o)
    res = run_bass_kernel_spmd(nc, in_maps, core_ids=list(range(NCORES)))

    out = np.zeros((B, S, D), np.float32)
    for core in range(NCORES):
        b = core // G
        # unpack the (quarter, row-block) tiled outputs
        oP = res.results[core]["outP"].reshape(128, NQC, KC, 512)
        out[b] += oP.transpose(2, 0, 1, 3).reshape(D, S).T.astype(np.float32)
        # pair-0 partial of the last quarter (p2-split tail)
        o2 = res.results[core]["out2P"].reshape(128, KC, 512)
        out[b, S - 512:S] += o2.transpose(1, 0, 2).reshape(D, 512).T.astype(np.float32)
    out += np.asarray(b_o, np.float32)
    return out


# revision 21
# speedup vs baseline: 1.0630x; 1.0630x over previous
"""Multi-head attention (B=2,S=2048,D=1024,H=16) on 8 trn2 NeuronCores.

Sharding: core = b*4 + g  (b = batch 0..1, g = head-group 0..3, 4 heads each).
Each core computes QKV projections for its 256 output dims, causal attention
for its 4 heads (scores kept transposed: [s_k, s_q]), and a K-sliced partial
of the output projection (transposed: [D, S]).  Host sums the 4 partials per
batch and adds b_o.

All matmuls in bf16 (fp32 PSUM accumulate); softmax without max-subtraction
(scores/8 are small, exp cannot overflow); sumexp via an all-ones [128,64]
stationary matmul that also broadcasts the sum to all partitions of each
head's half, so normalization is a plain elementwise multiply.

Pipeline structure: coarse per-quarter input DMAs spread over the sync /
scalar / gpsimd issue queues; PSUM split into dedicated pools (2x score
double-buffer, 2x projection, ctx, sumexp) so the score->exp->PV stream,
the woven QKV/O projections, and the accumulators never serialize through
a shared slot ring.  The last quarter's output projection is split per
head-pair (partial to a second output tensor summed on the host) to
shorten the kernel tail.
"""
import sys

if "/opt/trn_rl_repo" not in sys.path:
    sys.path.insert(0, "/opt/trn_rl_repo")

import numpy as np
import ml_dtypes

B, S, D, H = 2, 2048, 1024, 16
HD = D // H            # 64
G = 4                  # head groups (one per core within a batch)
HPG = H // G           # 4 heads per group
DG = HPG * HD          # 256 dims per group
SCALE = 8.0
NCORES = 8
NQC = S // 512         # 4 query chunks
NJ = S // 128          # 16 key tiles
KC = D // 128          # 8 contraction chunks
BF16 = ml_dtypes.bfloat16

_CACHE = {}


def _build(causal: bool):
    import concourse.mybir as mybir
    import concourse.tile as tile
    from concourse import bacc

    f32 = mybir.dt.float32
    b16 = mybir.dt.bfloat16
    Exp = mybir.ActivationFunctionType.Exp

    nc = bacc.Bacc(None, target_bir_lowering=False)

    # inputs host-prepacked quarter-major per partition: qP[p, n*KC*512 +
    # kc*512 + s] = q^T[kc*128 + p, n*512 + s] -- each quarter DMA is then a
    # plain 2D copy with 8KB-contiguous per-partition runs (fat DMA packets,
    # full HBM rate; strided 1KB packets lose the SDMA round-robin 4:1)
    qP = nc.dram_tensor("qP", [128, NQC * KC * 512], b16, kind="ExternalInput")
    kP = nc.dram_tensor("kP", [128, NQC * KC * 512], b16, kind="ExternalInput")
    vP = nc.dram_tensor("vP", [128, NQC * KC * 512], b16, kind="ExternalInput")
    # weights host-prepacked to the exact SBUF tile layout (one DMA each)
    wqT = nc.dram_tensor("wqT", [128, KC * DG], b16, kind="ExternalInput")
    wkT = nc.dram_tensor("wkT", [128, KC * DG], b16, kind="ExternalInput")
    wvT = nc.dram_tensor("wvT", [128, KC * DG], b16, kind="ExternalInput")
    woT = nc.dram_tensor("woT", [128, 2 * D], b16, kind="ExternalInput")
    bq = nc.dram_tensor("bq", [128, 2], f32, kind="ExternalInput")
    bk = nc.dram_tensor("bk", [128, 2], f32, kind="ExternalInput")
    bv128 = nc.dram_tensor("bv128", [128, DG], b16, kind="ExternalInput")
    tri = nc.dram_tensor("tri", [128, 128], b16, kind="ExternalInput")
    # output packed as contiguous (quarter, row-block) tiles so each
    # quarter's result leaves in ONE fat DMA: outP[p, (c*KC+dc)*512 + s] =
    # out[dc*128+p, c*512+s]; host unpacks (free)
    outP = nc.dram_tensor("outP", [128, NQC * KC * 512], b16, kind="ExternalOutput")
    # pair-0 partial of the last quarter's output projection (host adds it)
    out2P = nc.dram_tensor("out2P", [128, KC * 512], b16, kind="ExternalOutput")

    with tile.TileContext(nc) as tc:
        with (
            tc.tile_pool(name="consts", bufs=1) as consts,
            tc.tile_pool(name="proj", bufs=1) as proj,
            tc.tile_pool(name="pin", bufs=1) as pin,
            tc.tile_pool(name="probs", bufs=8) as probsp,
            tc.tile_pool(name="rec", bufs=2) as recp,
            tc.tile_pool(name="ost", bufs=1) as ostp,
            tc.tile_pool(name="scp", bufs=2, space="PSUM") as scp,
            tc.tile_pool(name="pp", bufs=2, space="PSUM") as ppool,
            tc.tile_pool(name="cpsum", bufs=1, space="PSUM") as cpsum,
            tc.tile_pool(name="upsum", bufs=1, space="PSUM") as upsum,
        ):
            # --- constant tiles -------------------------------------------
            wq_t = consts.tile([128, KC * DG], b16)
            wk_t = consts.tile([128, KC * DG], b16)
            wv_t = consts.tile([128, KC * DG], b16)
            wo_t = consts.tile([128, 2 * D], b16)
            bq_t = consts.tile([128, 2], f32)
            bk_t = consts.tile([128, 2], f32)
            bv_t = consts.tile([128, DG], b16)
            tri_t = consts.tile([128, 128], b16)
            ones64_t = consts.tile([128, HD], b16)
            nc.vector.memset(ones64_t[:], 1.0)
            warm_sb = consts.tile([128, 128], b16)
            nc.vector.memset(warm_sb[:], 0.0)

            # --- persistent projection outputs ----------------------------
            # qpT/kpT: pair p in cols [p*S,(p+1)*S); rows 0:64 head 2p, 64:128 head 2p+1
            qpT = proj.tile([128, 2 * S], b16)
            kpT = proj.tile([128, 2 * S], b16)
            # vp: key tile j in cols [j*DG,(j+1)*DG); within: local head hh at 64*hh
            vp = proj.tile([128, NJ * DG], b16)
            # ctxT: same pair layout as qpT, normalized attention output (c x s)
            ctxT = proj.tile([128, 2 * S], b16)

            # --- per-quarter input tiles + coarse DMA schedule ------------
            # quarter n of q/k/v lives in one tile, contraction chunk kc at
            # cols [kc*512,(kc+1)*512); one 1MB DMA per (tensor, quarter),
            # issued across three queues (sync / scalar / gpsimd) so first
            # bytes land fast and the HW rings stream at full HBM rate
            qin = [pin.tile([128, KC * 512], b16, name=f"qin{n}") for n in range(NQC)]
            kin = [pin.tile([128, KC * 512], b16, name=f"kin{n}") for n in range(NQC)]
            vin = [pin.tile([128, KC * 512], b16, name=f"vin{n}") for n in range(NQC)]

            def dma_quarter(eng, dst, srcP, n, split=1):
                w = KC * 512 // split
                for h in range(split):
                    eng.dma_start(dst[:, h * w:(h + 1) * w],
                                  srcP[:, n * KC * 512 + h * w: n * KC * 512 + (h + 1) * w])

            # ramp-critical transfers alternate across BOTH rings in
            # dependency-need order (each ring sustains only ~half the HBM
            # rate when both stream): wq+wk land together, then q0+k0, then
            # wv+v0.  The scalar queue carries NO DMAs: a blocked issue
            # there would stall the exp stream.
            nc.sync.dma_start(wq_t[:], wqT[:])
            nc.gpsimd.dma_start(wk_t[:], wkT[:])
            dma_quarter(nc.sync, qin[0], qP, 0, split=2)
            dma_quarter(nc.gpsimd, kin[0], kP, 0, split=2)
            nc.sync.dma_start(wv_t[:], wvT[:])
            dma_quarter(nc.gpsimd, vin[0], vP, 0)
            nc.gpsimd.dma_start(bv_t[:], bv128[:])
            nc.gpsimd.dma_start(tri_t[:], tri[:])
            nc.gpsimd.dma_start(bq_t[:], bq[:])
            nc.gpsimd.dma_start(bk_t[:], bk[:])
            nc.sync.dma_start(wo_t[:], woT[:])
            for n in range(1, NQC):
                dma_quarter(nc.sync, qin[n], qP, n)
                dma_quarter(nc.sync, kin[n], kP, n)
                dma_quarter(nc.gpsimd, vin[n], vP, n)

            # warmup burst: keeps the PE activity monitor at full clock
            # while the first input quarters stream in
            warm_ps = scp.tile([128, 1024], f32, tag="sc", name="warm")
            for wi in range(40):
                nc.tensor.matmul(warm_ps[:, 0:128], warm_sb[:], warm_sb[:],
                                 start=(wi == 0), stop=(wi == 39))
            # preload the exp spline tables (~2.7us) during the DMA window
            nc.scalar.activation(warm_sb[:, 0:1], warm_sb[:, 0:1], Exp)

            # --- projections interleaved with attention, per quarter ------

            def qk_proj_m(name, src_n, w_t, dst, bias_t, n, m):
                ps = ppool.tile([128, 512], f32, tag="pp", name=f"{name}ps{m}{n}")
                for kc in range(KC):
                    nc.tensor.matmul(
                        ps,
                        w_t[:, kc * DG + m * 128: kc * DG + (m + 1) * 128],
                        src_n[:, kc * 512:(kc + 1) * 512],
                        start=(kc == 0), stop=(kc == KC - 1),
                    )
                nc.vector.tensor_scalar_add(
                    dst[:, m * S + n * 512: m * S + (n + 1) * 512],
                    ps, bias_t[:, m:m + 1],
                )

            def qk_proj(name, srcs, w_t, dst, bias_t, n):
                for m in range(2):
                    qk_proj_m(name, srcs[n], w_t, dst, bias_t, n, m)

            def v_proj_j(j):
                n = j // 4
                jj = j - 4 * n
                ps = ppool.tile([128, 512], f32, tag="pp", name=f"vps{j}")
                for kc in range(KC):
                    nc.tensor.matmul(
                        ps[:, 0:DG],
                        vin[n][:, kc * 512 + jj * 128: kc * 512 + (jj + 1) * 128],
                        wv_t[:, kc * DG:(kc + 1) * DG],
                        start=(kc == 0), stop=(kc == KC - 1),
                    )
                # bias folded into the PSUM->SBUF move (the 1-partition bias
                # matmul forced a 32x128 tiling mode switch = PE drain)
                nc.vector.tensor_add(vp[:, j * DG:(j + 1) * DG], ps[:, 0:DG], bv_t[:])

            def v_proj(n):
                for j in range(4 * n, 4 * n + 4):
                    v_proj_j(j)

            def attn_j_sc(c, p, j, nj):
                qoff = p * S + c * 512
                d = j - 4 * c if causal else -1
                coff = 0 if d < 0 else 128 * d
                sc = scp.tile([128, 1024], f32, tag="sc", name=f"sc{c}{p}{j}")
                for hh, (rlo, rhi) in enumerate(((0, 64), (64, 128))):
                    nc.tensor.matmul(
                        sc[:, hh * 512 + coff: hh * 512 + 512],
                        kpT[rlo:rhi, p * S + j * 128: p * S + (j + 1) * 128],
                        qpT[rlo:rhi, qoff + coff: qoff + 512],
                        start=True, stop=True, tile_position=(rlo, 0),
                    )
                pr = probsp.tile([128, 1024], b16, tag="pr", name=f"pr{c}{p}{j}")
                if coff == 0:
                    nc.scalar.activation(pr[:, 0:1024], sc[:, 0:1024], Exp, scale=1.0 / SCALE)
                else:
                    sc_v = sc.rearrange("p (h n) -> p h n", h=2)[:, :, coff:512]
                    pr_v = pr.rearrange("p (h n) -> p h n", h=2)[:, :, coff:512]
                    nc.scalar.activation(pr_v, sc_v, Exp, scale=1.0 / SCALE)
                if d >= 0:
                    for hh in range(2):
                        band = pr[:, hh * 512 + coff: hh * 512 + coff + 128]
                        nc.vector.tensor_mul(band, band, tri_t[:])
                return pr

            def attn_j_pv(c, p, j, nj, ctx_ps, sum_ps, pr):
                d = j - 4 * c if causal else -1
                coff = 0 if d < 0 else 128 * d
                first, last = (j == 0), (j == nj - 1)
                for hh in range(2):
                    prh = pr[:, hh * 512 + coff: hh * 512 + 512]
                    nc.tensor.matmul(
                        ctx_ps[hh * 64:(hh + 1) * 64, coff:512],
                        vp[:, j * DG + p * 128 + hh * 64: j * DG + p * 128 + (hh + 1) * 64],
                        prh, start=first, stop=last,
                        tile_position=(0, hh * 64), skip_group_check=True,
                    )
                for hh in range(2):
                    prh = pr[:, hh * 512 + coff: hh * 512 + 512]
                    nc.tensor.matmul(
                        sum_ps[hh * 64:(hh + 1) * 64, coff:512],
                        ones64_t[:], prh, start=first, stop=last,
                        tile_position=(0, hh * 64), skip_group_check=True,
                    )

            def attn_j(c, p, j, nj, ctx_ps, sum_ps):
                pr = attn_j_sc(c, p, j, nj)
                attn_j_pv(c, p, j, nj, ctx_ps, sum_ps, pr)

            def attn_pair(c, p, nj, j_lo, j_hi, ctx_ps, sum_ps, bg=None, grp=4):
                # process j-tiles in groups: all scores (64x128 tiling mode)
                # then all PV+sum matmuls (128x64 mode) -- each mode change
                # drains the PE, so batching by mode cuts the switch count
                # ~4x.  bg: zero-arg projection emitters (128x128 mode)
                # woven at group boundaries to fill PE slack under the
                # ACT-paced softmax.
                bg = list(bg or [])
                js = list(range(j_lo, j_hi))
                n_groups = (len(js) + grp - 1) // grp
                for gi in range(n_groups):
                    gj = js[gi * grp:(gi + 1) * grp]
                    prs = [attn_j_sc(c, p, j, nj) for j in gj]
                    for j, pr in zip(gj, prs):
                        attn_j_pv(c, p, j, nj, ctx_ps, sum_ps, pr)
                    take = len(bg) // (n_groups - gi) if gi < n_groups - 1 else 0
                    for _ in range(take):
                        bg.pop(0)()
                while bg:
                    bg.pop(0)()

            def norm_pair(c, p, ctx_ps, sum_ps):
                rc_t = recp.tile([128, 512], f32, tag="rc", name=f"rc{c}{p}")
                nc.vector.reciprocal_approx_fast(rc_t[:], sum_ps[:])
                nc.vector.tensor_mul(ctxT[:, p * S + c * 512: p * S + (c + 1) * 512], ctx_ps[:], rc_t[:])

            ostage = {}

            def oproj_dc(c, dc):
                ops = ppool.tile([128, 512], f32, tag="pp", name=f"op{c}{dc}")
                for p2 in range(2):
                    nc.tensor.matmul(
                        ops,
                        wo_t[:, p2 * D + dc * 128: p2 * D + (dc + 1) * 128],
                        ctxT[:, p2 * S + c * 512: p2 * S + (c + 1) * 512],
                        start=(p2 == 0), stop=(p2 == 1),
                    )
                if dc == 0:
                    ostage[c] = ostp.tile([128, KC * 512], b16, tag="ostage", bufs=2, name=f"ost{c}")
                nc.vector.tensor_copy(ostage[c][:, dc * 512:(dc + 1) * 512], ops)
                if dc == KC - 1:
                    nc.sync.dma_start(outP[:, c * KC * 512:(c + 1) * KC * 512], ostage[c][:])

            # tail pools rotation: at the end of the kernel the attention
            # psum pools are free, so the last 8 oproj matmuls each get
            # their own slot and never wait on the staging copies
            _tailp = [(ppool, "pp"), (scp, "sc"), (cpsum, "ctx"), (upsum, "sum")]

            def oproj_half_dc(c, dc, p2, use_act=False):
                # one-pair partial of the output projection for quarter c.
                # p2==0 goes to out2P (host adds); p2==1 goes to outP.
                if p2 == 1:
                    pool, ptag = _tailp[dc % 4]
                else:
                    pool, ptag = ppool, "pp"
                ops = pool.tile([128, 512], f32, tag=ptag, name=f"oh{c}{dc}{p2}")
                nc.tensor.matmul(
                    ops,
                    wo_t[:, p2 * D + dc * 128: p2 * D + (dc + 1) * 128],
                    ctxT[:, p2 * S + c * 512: p2 * S + (c + 1) * 512],
                    start=True, stop=True,
                )
                if p2 == 0:
                    if dc == 0:
                        ostage["o2"] = ostp.tile([128, KC * 512], b16, tag="ost2", name="ost2")
                    nc.vector.tensor_copy(ostage["o2"][:, dc * 512:(dc + 1) * 512], ops)
                    if dc == KC - 1:
                        nc.sync.dma_start(out2P[:], ostage["o2"][:])
                else:
                    if dc == 0:
                        ostage[c] = ostp.tile([128, KC * 512], b16, tag="ostage", bufs=2, name=f"ost{c}")
                    dst = ostage[c][:, dc * 512:(dc + 1) * 512]
                    if use_act:
                        nc.scalar.copy(dst, ops)
                    else:
                        nc.vector.tensor_copy(dst, ops)
                    if dc == KC - 1:
                        nc.sync.dma_start(outP[:, c * KC * 512:(c + 1) * KC * 512], ostage[c][:])

            if not causal:
                # no diagonal structure to pipeline against: project all
                # quarters upfront
                qk_proj("q", qin, wq_t, qpT, bq_t, 0)
                for n in range(NQC):
                    if n > 0:
                        qk_proj("q", qin, wq_t, qpT, bq_t, n)
                    qk_proj("k", kin, wk_t, kpT, bk_t, n)
                    v_proj(n)
            for c in range(NQC):
                nj = 4 * c + 4 if causal else NJ
                ctx0 = cpsum.tile([128, 512], f32, tag="ctx", name=f"ctx{c}0")
                sum0 = upsum.tile([128, 512], f32, tag="sum", name=f"sum{c}0")
                if causal and c == 0:
                    # ramp: q/k projections and ALL c=0 scores first (they
                    # need only the q/k quarter-0 DMAs), v projections after
                    # (the v0 DMA lands while the exp stream spins up)
                    qk_proj_m("q", qin[0], wq_t, qpT, bq_t, 0, 0)
                    qk_proj_m("k", kin[0], wk_t, kpT, bk_t, 0, 0)
                    prs0 = [attn_j_sc(0, 0, j, nj) for j in range(nj)]
                    qk_proj_m("q", qin[0], wq_t, qpT, bq_t, 0, 1)
                    qk_proj_m("k", kin[0], wk_t, kpT, bk_t, 0, 1)
                    prs1 = [attn_j_sc(0, 1, j, nj) for j in range(nj)]
                    v_proj(0)
                    for j in range(nj):
                        attn_j_pv(0, 0, j, nj, ctx0, sum0, prs0[j])
                    norm_pair(0, 0, ctx0, sum0)
                    ctx1 = cpsum.tile([128, 512], f32, tag="ctx", name="ctx01")
                    sum1 = upsum.tile([128, 512], f32, tag="sum", name="sum01")
                    for j in range(nj):
                        attn_j_pv(0, 1, j, nj, ctx1, sum1, prs1[j])
                    qk_proj_m("q", qin[1], wq_t, qpT, bq_t, 1, 0)
                    qk_proj_m("q", qin[1], wq_t, qpT, bq_t, 1, 1)
                    norm_pair(0, 1, ctx1, sum1)
                    continue
                # part A (earlier-quarter key tiles; needs only qpT of this
                # quarter) with this quarter's k/v projections woven between
                # slots so they retire before the diagonal part B needs them
                bg_a = []
                if causal:
                    bg_a += [lambda m=m: qk_proj_m("k", kin[c], wk_t, kpT, bk_t, c, m) for m in range(2)]
                    bg_a += [lambda j=j: v_proj_j(j) for j in range(4 * c, 4 * c + 4)]
                attn_pair(c, 0, nj, 0, min(4 * c, nj), ctx0, sum0, bg=bg_a)
                attn_pair(c, 0, nj, min(4 * c, nj), nj, ctx0, sum0)
                norm_pair(c, 0, ctx0, sum0)
                # pair 1 hosts: previous chunk's output projection, the next
                # quarter's q projection, and (for the last quarter) the
                # pair-0 half of its own output projection
                bg_b = []
                if c > 0:
                    bg_b += [lambda dc=dc: oproj_dc(c - 1, dc) for dc in range(KC)]
                if causal and c + 1 < NQC:
                    bg_b += [lambda m=m: qk_proj_m("q", qin[c + 1], wq_t, qpT, bq_t, c + 1, m) for m in range(2)]
                if c == NQC - 1:
                    bg_b += [lambda dc=dc: oproj_half_dc(c, dc, 0) for dc in range(KC)]
                ctx1 = cpsum.tile([128, 512], f32, tag="ctx", name=f"ctx{c}1")
                sum1 = upsum.tile([128, 512], f32, tag="sum", name=f"sum{c}1")
                attn_pair(c, 1, nj, 0, nj, ctx1, sum1, bg=bg_b)
                norm_pair(c, 1, ctx1, sum1)
            # tail: only the pair-1 half of the last quarter's output
            # projection remains; copies alternate DVE/ACT (ACT is idle now)
            for dc in range(KC):
                oproj_half_dc(NQC - 1, dc, 1, use_act=(dc % 2 == 1))

    nc.compile()
    return nc


def _get_nc(causal: bool):
    if causal not in _CACHE:
        _CACHE[causal] = _build(causal)
    return _CACHE[causal]


def _pack_w(w):
    # [D, DG] -> SBUF layout [128, KC*DG]: chunk kc of 128 rows side by side
    return np.ascontiguousarray(w.reshape(KC, 128, DG).transpose(1, 0, 2).reshape(128, KC * DG)).astype(BF16)


def _pack_in(x):
    # x [S, D] -> xP [128, NQC*KC*512]: xP[p, (n, kc, s)] = x^T[kc*128+p, n*512+s]
    xT = x.T.reshape(KC, 128, NQC, 512)
    return np.ascontiguousarray(xT.transpose(1, 2, 0, 3).reshape(128, NQC * KC * 512)).astype(BF16)


def make_in_maps(q, k, v, w_q, b_q, w_k, b_k, w_v, b_v, w_o):
    tri_keep = (np.arange(128)[:, None] <= np.arange(128)[None, :]).astype(BF16)
    qPn = [_pack_in(q[b]) for b in range(B)]
    kPn = [_pack_in(k[b]) for b in range(B)]
    vPn = [_pack_in(v[b]) for b in range(B)]
    in_maps = []
    for core in range(NCORES):
        b, g = core // G, core % G
        sl = slice(g * DG, (g + 1) * DG)
        woTg = np.ascontiguousarray(w_o[:, sl].T)  # [DG, D]
        in_maps.append({
            "qP": qPn[b], "kP": kPn[b], "vP": vPn[b],
            "wqT": _pack_w(np.ascontiguousarray(w_q[sl, :].T)),
            "wkT": _pack_w(np.ascontiguousarray(w_k[sl, :].T)),
            "wvT": _pack_w(np.ascontiguousarray(w_v[sl, :].T)),
            "woT": np.ascontiguousarray(
                woTg.reshape(2, 128, D).transpose(1, 0, 2).reshape(128, 2 * D)).astype(BF16),
            "bq": np.ascontiguousarray(b_q[sl].reshape(2, 128).T).astype(np.float32),
            "bk": np.ascontiguousarray(b_k[sl].reshape(2, 128).T).astype(np.float32),
            "bv128": np.ascontiguousarray(np.broadcast_to(b_v[None, sl], (128, DG))).astype(BF16),
            "tri": tri_keep,
        })
    return in_maps


def _reference_numpy(q, k, v, mask, w_q, b_q, w_k, b_k, w_v, b_v, w_o, b_o):
    qp = q @ w_q.T + b_q
    kp = k @ w_k.T + b_k
    vv = v @ w_v.T + b_v
    qp = qp.reshape(B, S, H, HD).transpose(0, 2, 1, 3)
    kp = kp.reshape(B, S, H, HD).transpose(0, 2, 1, 3)
    vv = vv.reshape(B, S, H, HD).transpose(0, 2, 1, 3)
    score = np.einsum("bhqd,bhkd->bhqk", qp, kp) / SCALE
    score = np.where(mask, -1e9, score)
    score -= score.max(axis=-1, keepdims=True)
    e = np.exp(score)
    attn = e / e.sum(axis=-1, keepdims=True)
    ctx = np.einsum("bhqk,bhkd->bhqd", attn, vv)
    ctx = ctx.transpose(0, 2, 1, 3).reshape(B, S, D)
    return (ctx @ w_o.T + b_o).astype(np.float32)


def kernel(q, k, v, mask, w_q, b_q, w_k, b_k, w_v, b_v, w_o, b_o):
    from concourse.bass_utils import run_bass_kernel_spmd

    q, k, v = (np.asarray(x, np.float32) for x in (q, k, v))
    mask = np.asarray(mask)
    causal_ref = np.triu(np.ones((S, S), bool), k=1)
    causal = all(np.array_equal(mask[b, 0], causal_ref) for b in range(B))
    if not causal and mask.any():
        # Unexpected mask pattern: fall back to exact numpy (never hit in
        # practice -- setup_inputs always builds the causal mask).
        return _reference_numpy(q, k, v, mask, w_q, b_q, w_k, b_k, w_v, b_v, w_o, b_o)

    nc = _get_nc(causal)
    in_maps = make_in_maps(q, k, v, w_q, b_q, w_k, b_k, w_v, b_v, w_o)
    res = run_bass_kernel_spmd(nc, in_maps, core_ids=list(range(NCORES)))

    out = np.zeros((B, S, D), np.float32)
    for core in range(NCORES):
        b = core // G
        # unpack the (quarter, row-block) tiled outputs
        oP = res.results[core]["outP"].reshape(128, NQC, KC, 512)
        out[b] += oP.transpose(2, 0, 1, 3).reshape(D, S).T.astype(np.float32)
        # pair-0 partial of the last quarter (p2-split tail)
        o2 = res.results[core]["out2P"].reshape(128, KC, 512)
        out[b, S - 512:S] += o2.transpose(1, 0, 2).reshape(D, 512).T.astype(np.float32)
    out += np.asarray(b_o, np.float32)
    return out
